# revision 47
# baseline (speedup 1.0000x reference)
"""GATv2 attention-pool kernel for 8 Trainium2 NeuronCores.

Algorithm
---------
Reference computes, per edge e with target node t(e):
    feats = q + k                                   [E, 64]
    logits[e,h] = sum_c feats[e,h*8+c] * A[c,h]     [E, 8]
    attn = segment_softmax(logits, targets)         [E, 8]
    out[n] = relu(segment_sum(q * attn))            [N, 64]

Because logits are O(20), exp() never overflows fp32/bf16, so the
segment-max shift is unnecessary and softmax folds into two segment-SUMS:
    denom[n,h]  = sum_{e->n} exp(logits[e,h])
    pooled[n,:] = sum_{e->n} q[e,:] * exp(logits[e,h])
    out[n]      = relu(pooled[n] / denom[n])

Distribution: edges are partitioned by target node (host-side sort), 100000
nodes split into 8 contiguous shards of 12500 -> all segment reductions are
core-local, no collectives.  A deficit-steering packer bins each shard's
nodes into windows of <= 32 nodes whose edge totals land within a few edges
of the TSUB*128 slot capacity (99.6% fill); G windows form a group brought
in by one contiguous DMA (14KB/partition descriptors).

Datapath is fp16 (bf16 for exp outputs, f32 psum accumulation):
 - the one-hot selector S (pure index data) is precomputed on the host and
   streamed as fp8 so the vector engine never builds it;
 - logits fold (q+k)*w with a 3-level binary tree of tensor_tensor adds
   (the fastest DVE op shape on this hardware);
 - per 128-edge subtile the PE accumulates psum += S^T @ [q*ex | ex]
   (fp8 x bf16, 4x faster than fp32 matmul); two 32-node windows stack
   into the 64 psum partitions, then the epilogue drains psum through the
   scalar engine, divides and relus once per node.

Host work is index metadata + data layout only (argsort of targets, gather
of q/k rows into slot order, fp16 packing, one-hot expansion of the
relative node index); all floating-point arithmetic runs on device.
"""

import os
import sys

import numpy as np

N_NODES = 100000
N_EDGES = 1600000
H = 8
C = 8
HC = H * C
MW = 2 * HC
N_CORES = 8
NODES_PER_CORE = N_NODES // N_CORES
WIN_NODES = 32
SUB = 128
TSUB = 4          # subtiles per window (window edge capacity = TSUB*128)
G = 14            # windows per group (one DMA + one DVE pass per group)
PJ = HC + H       # psum cols per window: 64 numerator + 8 denominator


def _ensure_imports():
    try:
        import concourse.bass  # noqa: F401
    except ImportError:
        for p in ("/opt/trn_rl_repo", "/root/.axon_site/_ro/trn_rl_repo"):
            if os.path.isdir(p) and p not in sys.path:
                sys.path.insert(0, p)


def pack_windows(targets):
    """Sort edges by target; bin each core's nodes into windows.

    Deficit steering: every window tracks the remaining edge budget and
    repeatedly takes the unplaced node whose degree is closest to
    budget/slots-left, so windows land within a few edges of cap_e.
    """
    import bisect

    order = np.argsort(targets, kind="stable")
    tsorted = targets[order]
    node_start = np.searchsorted(tsorted, np.arange(N_NODES + 1))
    deg = np.diff(node_start)

    cap_e = TSUB * SUB
    packs = []
    for c in range(N_CORES):
        base = c * NODES_PER_CORE
        pairs = sorted((int(deg[base + i]), base + i)
                       for i in range(NODES_PER_CORE))
        keys = [p[0] for p in pairs]
        vals = [p[1] for p in pairs]
        wins = []
        while keys:
            cur, cnt = [], 0
            while keys and len(cur) < WIN_NODES and cnt < cap_e:
                need = (cap_e - cnt) / (WIN_NODES - len(cur))
                i = bisect.bisect_left(keys, need)
                if i >= len(keys):
                    i = len(keys) - 1
                elif i > 0 and keys[i] > cap_e - cnt:
                    i -= 1
                if keys[i] > cap_e - cnt:
                    break
                cur.append(vals[i])
                cnt += keys[i]
                del keys[i], vals[i]
            wins.append(cur)
        packs.append(wins)
    return packs, order, node_start


def build_slots(packs, order, node_start):
    """Lay out edge slots in DRAM row order r = gbase + p*Tg + t.

    Window wi of a group owns subtile columns [wi*TSUB, (wi+1)*TSUB); within
    a window, edge j sits at (p = j // TSUB, t_local = j % TSUB), so each
    node's slot run is contiguous through the group-slab view.
    """
    n_win = max(len(w) for w in packs)
    # small warmup/cooldown groups shrink the pipeline fill (DVE idles
    # until the first group's DMA lands) and drain (last matmul+epilogue)
    group_sizes = []
    rem = n_win
    for s in (4, 10):
        if rem > G + s:
            group_sizes.append(s)
            rem -= s
    tail = []
    for s in (4, 10):
        if rem > G + s:
            tail.append(s)
            rem -= s
    while rem > 0:
        s = min(G, rem)
        group_sizes.append(s)
        rem -= s
    group_sizes.extend(reversed(tail))
    n_slots = sum(128 * gs * TSUB for gs in group_sizes)

    perms = np.zeros((N_CORES, n_slots), dtype=np.int64)
    rels = np.full((N_CORES, n_slots), -1, dtype=np.int16)
    node_order = np.full((N_CORES, n_win * WIN_NODES), -1, dtype=np.int64)
    for c in range(N_CORES):
        gbase = 0
        wbase = 0
        for gi, gs in enumerate(group_sizes):
            tg = gs * TSUB
            pslab = perms[c, gbase:gbase + 128 * tg].reshape(128, tg)
            rslab = rels[c, gbase:gbase + 128 * tg].reshape(128, tg)
            for wl in range(gs):
                w = wbase + wl
                if w >= len(packs[c]):
                    continue
                pw = pslab[:, wl * TSUB:(wl + 1) * TSUB]
                rw = rslab[:, wl * TSUB:(wl + 1) * TSUB]
                pos = 0
                for j, node in enumerate(packs[c][w]):
                    e0, e1 = node_start[node], node_start[node + 1]
                    cnt = e1 - e0
                    # .flat writes through the non-contiguous column view
                    pw.flat[pos:pos + cnt] = order[e0:e1]
                    rw.flat[pos:pos + cnt] = j
                    pos += cnt
                    node_order[c, w * WIN_NODES + j] = node
            gbase += 128 * tg
            wbase += gs
    return perms, rels, node_order, n_win, n_slots, group_sizes


def build_nc(n_slots, n_win, group_sizes):
    """Build the single SPMD Bass program for one core's shard."""
    _ensure_imports()
    import concourse.bacc as bacc
    import concourse.mybir as mybir
    import concourse.tile as tile

    f32 = mybir.dt.float32
    f16 = mybir.dt.float16
    bf16 = mybir.dt.bfloat16
    f8 = mybir.dt.float8e4

    OP = mybir.AluOpType
    AF = mybir.ActivationFunctionType

    # two 32-node windows stack into 64 psum partitions; nb = column blocks
    nbs = [(gs + 1) // 2 for gs in group_sizes]
    nb_total = sum(nbs)

    nc = bacc.Bacc("TRN2", num_devices=N_CORES)
    qk = nc.declare_dram_parameter("qk", [n_slots, MW], f16, False)
    sel = nc.declare_dram_parameter("sel", [n_slots, WIN_NODES], f8, False)
    wrow = nc.declare_dram_parameter("wrow", [128, HC], f16, False)
    out = nc.declare_dram_parameter("out", [2 * WIN_NODES, nb_total * HC],
                                    f32, isOutput=True)

    with tile.TileContext(nc) as tc:
        with (
            tc.tile_pool(name="const", bufs=1) as cpool,
            tc.tile_pool(name="qk", bufs=5) as qkpool,
            tc.tile_pool(name="sel", bufs=5) as selpool,
            tc.tile_pool(name="mid", bufs=2) as midpool,
            tc.tile_pool(name="lgp", bufs=4) as lgpool,
            tc.tile_pool(name="mt", bufs=3) as mtpool,
            tc.tile_pool(name="fin", bufs=3) as finpool,
            tc.tile_pool(name="psum", bufs=8, space="PSUM") as ppool,
        ):
            w_t = cpool.tile([128, HC], f16)
            nc.sync.dma_start(out=w_t[:], in_=wrow[:])

            def emit_load(gi, gbase, gs):
                tg = gs * TSUB
                nsl = 128 * tg
                qk_t = qkpool.tile([128, tg, MW], f16, tag="qk")
                nc.sync.dma_start(
                    out=qk_t[:],
                    in_=qk[gbase:gbase + nsl, :].rearrange(
                        "(p t) c -> p t c", p=128),
                )
                s_t = selpool.tile([128, tg, WIN_NODES], f8, tag="S")
                nc.sync.dma_start(
                    out=s_t[:],
                    in_=sel[gbase:gbase + nsl, :].rearrange(
                        "(p t) n -> p t n", p=128),
                )
                return {"gi": gi, "gs": gs, "tg": tg, "qk": qk_t,
                        "S": s_t}

            def emit_logits(s):
                tg = s["tg"]
                qk3 = s["qk"]
                # f = q + k   (tensor_tensor is the fastest DVE op on hw)
                f_t = midpool.tile([128, tg, HC], f16, tag="f")
                nc.vector.tensor_tensor(
                    out=f_t[:], in0=qk3[:, :, 0:HC],
                    in1=qk3[:, :, HC:MW], op=OP.add)
                # wf = f * w  (w broadcast over t: middle dim)
                wf_t = midpool.tile([128, tg, HC], f16, tag="wf")
                nc.vector.tensor_tensor(
                    out=wf_t[:], in0=f_t[:],
                    in1=w_t[:, None, :].to_broadcast([128, tg, HC]),
                    op=OP.mult)
                # binary-tree reduce over c (c-major layout: col c*8+h)
                t1 = midpool.tile([128, tg, HC // 2], f16, tag="t1")
                nc.vector.tensor_tensor(
                    out=t1[:], in0=wf_t[:, :, 0:32],
                    in1=wf_t[:, :, 32:64], op=OP.add)
                t2 = midpool.tile([128, tg, HC // 4], f16, tag="t2")
                nc.vector.tensor_tensor(
                    out=t2[:], in0=t1[:, :, 0:16],
                    in1=t1[:, :, 16:32], op=OP.add)
                lg = lgpool.tile([128, tg, H], f16, tag="lg")
                nc.vector.tensor_tensor(
                    out=lg[:], in0=t2[:, :, 0:8],
                    in1=t2[:, :, 8:16], op=OP.add)
                s["lg"] = lg

            def emit_m(s):
                tg = s["tg"]
                m_t = mtpool.tile([128, tg, PJ], bf16, tag="M")
                nc.scalar.activation(
                    out=m_t[:, :, HC:PJ], in_=s["lg"][:], func=AF.Exp)
                # m = q * ex (ex broadcast over c: middle dim, packed h last)
                nc.vector.tensor_tensor(
                    out=m_t[:, :, 0:HC].rearrange(
                        "p t (c h) -> p t c h", h=H),
                    in0=s["qk"][:, :, 0:HC].rearrange(
                        "p t (c h) -> p t c h", h=H),
                    in1=m_t[:, :, HC:PJ][:, :, None, :].to_broadcast(
                        [128, tg, C, H]),
                    op=OP.mult)
                s["m"] = m_t

            def emit_mm(s):
                gs = s["gs"]
                nb = (gs + 1) // 2
                p_t = ppool.tile([2 * WIN_NODES, nb * PJ], f32)
                for wl in range(gs):
                    x, b = wl % 2, wl // 2
                    prow = slice(x * WIN_NODES, (x + 1) * WIN_NODES)
                    pcols = slice(b * PJ, (b + 1) * PJ)
                    for g in range(TSUB):
                        t = wl * TSUB + g
                        nc.tensor.matmul(
                            p_t[prow, pcols],
                            lhsT=s["S"][:, t, :],
                            rhs=s["m"][:, t, :],
                            start=(g == 0),
                            stop=(g == TSUB - 1),
                        )
                s["psum"] = p_t

            def emit_drain(s):
                gs = s["gs"]
                nb = (gs + 1) // 2
                # drain psum through the (idle) scalar engine, folding the
                # (c,h) -> (h,c) permute into the numerator copy so the
                # vector ops stream contiguous SBUF
                p3 = s["psum"][:].rearrange("p (w j) -> p w j", j=PJ)
                ppn = finpool.tile([2 * WIN_NODES, nb, H, C], f32,
                                   tag="ppn")
                nc.scalar.activation(
                    out=ppn[:].rearrange("p w h c -> p w c h"),
                    in_=p3[:, :, 0:HC].rearrange(
                        "p w (c h) -> p w c h", h=H),
                    func=AF.Copy)
                ppd = finpool.tile([2 * WIN_NODES, nb, H], f32, tag="ppd")
                nc.scalar.activation(
                    out=ppd[:], in_=p3[:, :, HC:PJ], func=AF.Copy)
                s["ppn"], s["ppd"] = ppn, ppd
                s["psum"] = None

            def emit_finish(s, bbase):
                gs = s["gs"]
                nb = (gs + 1) // 2
                rc_t = finpool.tile([2 * WIN_NODES, nb, H], f32, tag="rc")
                nc.vector.reciprocal(rc_t[:], s["ppd"][:])
                d_t = finpool.tile([2 * WIN_NODES, nb, H, C], f32, tag="d")
                nc.vector.tensor_tensor(
                    out=d_t[:],
                    in0=s["ppn"][:],
                    in1=rc_t[:, :, :, None].to_broadcast(
                        [2 * WIN_NODES, nb, H, C]),
                    op=OP.mult)
                o_t = finpool.tile([2 * WIN_NODES, nb, HC], f32, tag="o")
                nc.scalar.activation(
                    o_t[:], d_t[:].rearrange("p w h c -> p w (h c)"),
                    func=AF.Relu)
                nc.sync.dma_start(
                    out=out[:, bbase * HC:(bbase + nb) * HC].rearrange(
                        "p (w c) -> p w c", c=HC),
                    in_=o_t[:],
                )

            gbases, bbases = [], []
            gb = bb = 0
            for gs, nb in zip(group_sizes, nbs):
                gbases.append(gb)
                bbases.append(bb)
                gb += 128 * gs * TSUB
                bb += nb

            ng = len(group_sizes)

            def load(i):
                return emit_load(i, gbases[i], group_sizes[i])

            # software pipeline with 3-group lookahead: the epilogue of
            # group i is emitted two iterations after its matmuls, loads
            # run three ahead, so the DVE always has logits work queued
            # while the PE finishes earlier groups
            st = [None] * ng
            st[0] = load(0)
            emit_logits(st[0])
            if ng > 1:
                st[1] = load(1)
            if ng > 2:
                st[2] = load(2)
            for i in range(ng):
                if i + 3 < ng:
                    st[i + 3] = load(i + 3)
                emit_m(st[i])
                emit_mm(st[i])
                if i + 1 < ng:
                    emit_logits(st[i + 1])
                if i >= 1:
                    emit_drain(st[i - 1])
                if i >= 2:
                    emit_finish(st[i - 2], bbases[i - 2])
                    st[i - 2] = None
            for i in range(max(0, ng - 2), ng):
                if st[i] is not None and st[i].get("ppn") is None:
                    emit_drain(st[i])
                emit_finish(st[i], bbases[i])

    nc.finalize()
    return nc


def _host_arrays(query, key, attn_kernel, targets):
    import ml_dtypes

    packs, order, node_start = pack_windows(targets)
    perms, rels, node_order, n_win, n_slots, group_sizes = build_slots(
        packs, order, node_start)

    # (c,h)-major column permutation: new col c*8+h <- orig col h*8+c
    colperm = (np.arange(HC).reshape(C, H).T).reshape(-1)
    wrow_1 = np.asarray(attn_kernel, dtype=np.float16).reshape(-1)  # A[c,h]
    wrow = np.tile(wrow_1, (128, 1))

    q16 = query[:, colperm].astype(np.float16)
    k16 = key[:, colperm].astype(np.float16)
    iota = np.arange(WIN_NODES, dtype=np.int16)
    in_maps = []
    for c in range(N_CORES):
        qkc = np.zeros((n_slots, MW), dtype=np.float16)
        pc = perms[c]
        used = rels[c] >= 0
        qkc[used, :HC] = q16[pc[used]]
        qkc[used, HC:] = k16[pc[used]]
        # one-hot selector in fp8 (1.0 = 0x38): pure index metadata
        sel_u8 = np.where(rels[c][:, None] == iota[None, :],
                          np.uint8(0x38), np.uint8(0))
        in_maps.append({
            "qk": qkc,
            "sel": sel_u8.view(ml_dtypes.float8_e4m3),
            "wrow": wrow,
        })
    return in_maps, node_order, n_win, n_slots, group_sizes


TRACE = False          # set by test harness to capture an NTFF profile
TRACE_CORES = None
LAST_RESULTS = None    # BassKernelResults of the most recent run


def kernel(query, key, attn_kernel, targets):
    global LAST_RESULTS
    query = np.asarray(query, dtype=np.float32)
    key = np.asarray(key, dtype=np.float32)
    attn_kernel = np.asarray(attn_kernel, dtype=np.float32)
    targets = np.asarray(targets, dtype=np.int32)

    _ensure_imports()
    from concourse.bass_utils import run_bass_kernel_spmd

    in_maps, node_order, n_win, n_slots, group_sizes = _host_arrays(
        query, key, attn_kernel, targets)
    nc = build_nc(n_slots, n_win, group_sizes)
    res = run_bass_kernel_spmd(
        nc, in_maps, list(range(N_CORES)),
        trace=TRACE, trace_cores=TRACE_CORES,
    )
    LAST_RESULTS = res

    # unscramble: psum partition-half x + column block b -> window w
    out = np.zeros((N_NODES, HC), dtype=np.float32)
    nbs = [(gs + 1) // 2 for gs in group_sizes]
    for c in range(N_CORES):
        oc = res.results[c]["out"]  # [64, nb_total*HC]
        bb = 0
        wbase = 0
        for gi, (gs, nb) in enumerate(zip(group_sizes, nbs)):
            for b in range(nb):
                for x in range(2):
                    w = wbase + b * 2 + x
                    if w >= wbase + gs:
                        continue
                    rows = node_order[c, w * WIN_NODES:(w + 1) * WIN_NODES]
                    valid = rows >= 0
                    blk = oc[x * WIN_NODES:(x + 1) * WIN_NODES,
                             (bb + b) * HC:(bb + b + 1) * HC]
                    out[rows[valid]] = blk[valid]
            bb += nb
            wbase += gs

    deg = np.bincount(targets, minlength=N_NODES)
    out[deg == 0] = 0.0
    return out


# revision 48
# speedup vs baseline: 1.0067x; 1.0067x over previous
"""GATv2 attention-pool kernel for 8 Trainium2 NeuronCores.

Algorithm
---------
Reference computes, per edge e with target node t(e):
    feats = q + k                                   [E, 64]
    logits[e,h] = sum_c feats[e,h*8+c] * A[c,h]     [E, 8]
    attn = segment_softmax(logits, targets)         [E, 8]
    out[n] = relu(segment_sum(q * attn))            [N, 64]

Because logits are O(20), exp() never overflows fp32/bf16, so the
segment-max shift is unnecessary and softmax folds into two segment-SUMS:
    denom[n,h]  = sum_{e->n} exp(logits[e,h])
    pooled[n,:] = sum_{e->n} q[e,:] * exp(logits[e,h])
    out[n]      = relu(pooled[n] / denom[n])

Distribution: edges are partitioned by target node (host-side sort), 100000
nodes split into 8 contiguous shards of 12500 -> all segment reductions are
core-local, no collectives.  A deficit-steering packer bins each shard's
nodes into windows of <= 32 nodes whose edge totals land within a few edges
of the TSUB*128 slot capacity (99.6% fill); G windows form a group brought
in by one contiguous DMA (14KB/partition descriptors).

Datapath is fp16 (bf16 for exp outputs, f32 psum accumulation):
 - the one-hot selector S (pure index data) is precomputed on the host and
   streamed as fp8 so the vector engine never builds it;
 - logits fold (q+k)*w with a 3-level binary tree of tensor_tensor adds
   (the fastest DVE op shape on this hardware);
 - per 128-edge subtile the PE accumulates psum += S^T @ [q*ex | ex]
   (fp8 x bf16, 4x faster than fp32 matmul); two 32-node windows stack
   into the 64 psum partitions, then the epilogue drains psum through the
   scalar engine, divides and relus once per node.

Host work is index metadata + data layout only (argsort of targets, gather
of q/k rows into slot order, fp16 packing, one-hot expansion of the
relative node index); all floating-point arithmetic runs on device.
"""

import os
import sys

import numpy as np

N_NODES = 100000
N_EDGES = 1600000
H = 8
C = 8
HC = H * C
MW = 2 * HC
N_CORES = 8
NODES_PER_CORE = N_NODES // N_CORES
WIN_NODES = 32
SUB = 128
TSUB = 4          # subtiles per window (window edge capacity = TSUB*128)
G = 14            # windows per group (one DMA + one DVE pass per group)
PJ = HC + H       # psum cols per window: 64 numerator + 8 denominator


def _ensure_imports():
    try:
        import concourse.bass  # noqa: F401
    except ImportError:
        for p in ("/opt/trn_rl_repo", "/root/.axon_site/_ro/trn_rl_repo"):
            if os.path.isdir(p) and p not in sys.path:
                sys.path.insert(0, p)


def pack_windows(targets):
    """Sort edges by target; bin each core's nodes into windows.

    Deficit steering: every window tracks the remaining edge budget and
    repeatedly takes the unplaced node whose degree is closest to
    budget/slots-left, so windows land within a few edges of cap_e.
    """
    import bisect

    order = np.argsort(targets, kind="stable")
    tsorted = targets[order]
    node_start = np.searchsorted(tsorted, np.arange(N_NODES + 1))
    deg = np.diff(node_start)

    cap_e = TSUB * SUB
    packs = []
    for c in range(N_CORES):
        base = c * NODES_PER_CORE
        pairs = sorted((int(deg[base + i]), base + i)
                       for i in range(NODES_PER_CORE))
        keys = [p[0] for p in pairs]
        vals = [p[1] for p in pairs]
        wins = []
        while keys:
            cur, cnt = [], 0
            while keys and len(cur) < WIN_NODES and cnt < cap_e:
                need = (cap_e - cnt) / (WIN_NODES - len(cur))
                i = bisect.bisect_left(keys, need)
                if i >= len(keys):
                    i = len(keys) - 1
                elif i > 0 and keys[i] > cap_e - cnt:
                    i -= 1
                if keys[i] > cap_e - cnt:
                    break
                cur.append(vals[i])
                cnt += keys[i]
                del keys[i], vals[i]
            wins.append(cur)
        packs.append(wins)
    return packs, order, node_start


def build_slots(packs, order, node_start):
    """Lay out edge slots in DRAM row order r = gbase + p*Tg + t.

    Window wi of a group owns subtile columns [wi*TSUB, (wi+1)*TSUB); within
    a window, edge j sits at (p = j // TSUB, t_local = j % TSUB), so each
    node's slot run is contiguous through the group-slab view.
    """
    n_win = max(len(w) for w in packs)
    # small warmup/cooldown groups shrink the pipeline fill (DVE idles
    # until the first group's DMA lands) and drain (last matmul+epilogue)
    group_sizes = []
    rem = n_win
    for s in (4, 10):
        if rem > G + s:
            group_sizes.append(s)
            rem -= s
    tail = []
    for s in (4, 10):
        if rem > G + s:
            tail.append(s)
            rem -= s
    while rem > 0:
        s = min(G, rem)
        group_sizes.append(s)
        rem -= s
    group_sizes.extend(reversed(tail))
    n_slots = sum(128 * gs * TSUB for gs in group_sizes)

    perms = np.zeros((N_CORES, n_slots), dtype=np.int64)
    rels = np.full((N_CORES, n_slots), -1, dtype=np.int16)
    node_order = np.full((N_CORES, n_win * WIN_NODES), -1, dtype=np.int64)
    for c in range(N_CORES):
        gbase = 0
        wbase = 0
        for gi, gs in enumerate(group_sizes):
            tg = gs * TSUB
            pslab = perms[c, gbase:gbase + 128 * tg].reshape(128, tg)
            rslab = rels[c, gbase:gbase + 128 * tg].reshape(128, tg)
            for wl in range(gs):
                w = wbase + wl
                if w >= len(packs[c]):
                    continue
                pw = pslab[:, wl * TSUB:(wl + 1) * TSUB]
                rw = rslab[:, wl * TSUB:(wl + 1) * TSUB]
                pos = 0
                for j, node in enumerate(packs[c][w]):
                    e0, e1 = node_start[node], node_start[node + 1]
                    cnt = e1 - e0
                    # .flat writes through the non-contiguous column view
                    pw.flat[pos:pos + cnt] = order[e0:e1]
                    rw.flat[pos:pos + cnt] = j
                    pos += cnt
                    node_order[c, w * WIN_NODES + j] = node
            gbase += 128 * tg
            wbase += gs
    return perms, rels, node_order, n_win, n_slots, group_sizes


def build_nc(n_slots, n_win, group_sizes):
    """Build the single SPMD Bass program for one core's shard."""
    _ensure_imports()
    import concourse.bacc as bacc
    import concourse.mybir as mybir
    import concourse.tile as tile

    f32 = mybir.dt.float32
    f16 = mybir.dt.float16
    bf16 = mybir.dt.bfloat16
    f8 = mybir.dt.float8e4

    OP = mybir.AluOpType
    AF = mybir.ActivationFunctionType

    # two 32-node windows stack into 64 psum partitions; nb = column blocks
    nbs = [(gs + 1) // 2 for gs in group_sizes]
    nb_total = sum(nbs)

    nc = bacc.Bacc("TRN2", num_devices=N_CORES)
    qk = nc.declare_dram_parameter("qk", [n_slots, MW], f16, False)
    sel = nc.declare_dram_parameter("sel", [n_slots, WIN_NODES], f8, False)
    wrow = nc.declare_dram_parameter("wrow", [128, HC], f16, False)
    out = nc.declare_dram_parameter("out", [2 * WIN_NODES, nb_total * HC],
                                    f32, isOutput=True)

    with tile.TileContext(nc) as tc:
        with (
            tc.tile_pool(name="const", bufs=1) as cpool,
            tc.tile_pool(name="qk", bufs=5) as qkpool,
            tc.tile_pool(name="sel", bufs=5) as selpool,
            tc.tile_pool(name="mid", bufs=2) as midpool,
            tc.tile_pool(name="lgp", bufs=4) as lgpool,
            tc.tile_pool(name="mt", bufs=3) as mtpool,
            tc.tile_pool(name="fin", bufs=3) as finpool,
            tc.tile_pool(name="psum", bufs=8, space="PSUM") as ppool,
        ):
            w_t = cpool.tile([128, HC], f16)
            nc.sync.dma_start(out=w_t[:], in_=wrow[:])

            def emit_load(gi, gbase, gs):
                tg = gs * TSUB
                nsl = 128 * tg
                qk_t = qkpool.tile([128, tg, MW], f16, tag="qk")
                nc.sync.dma_start(
                    out=qk_t[:],
                    in_=qk[gbase:gbase + nsl, :].rearrange(
                        "(p t) c -> p t c", p=128),
                )
                s_t = selpool.tile([128, tg, WIN_NODES], f8, tag="S")
                nc.sync.dma_start(
                    out=s_t[:],
                    in_=sel[gbase:gbase + nsl, :].rearrange(
                        "(p t) n -> p t n", p=128),
                )
                return {"gi": gi, "gs": gs, "tg": tg, "qk": qk_t,
                        "S": s_t}

            def emit_logits(s):
                tg = s["tg"]
                qk3 = s["qk"]
                # f = q + k   (tensor_tensor is the fastest DVE op on hw)
                f_t = midpool.tile([128, tg, HC], f16, tag="f")
                nc.vector.tensor_tensor(
                    out=f_t[:], in0=qk3[:, :, 0:HC],
                    in1=qk3[:, :, HC:MW], op=OP.add)
                # wf = f * w  (w broadcast over t: middle dim)
                wf_t = midpool.tile([128, tg, HC], f16, tag="wf")
                nc.vector.tensor_tensor(
                    out=wf_t[:], in0=f_t[:],
                    in1=w_t[:, None, :].to_broadcast([128, tg, HC]),
                    op=OP.mult)
                # binary-tree reduce over c (c-major layout: col c*8+h)
                t1 = midpool.tile([128, tg, HC // 2], f16, tag="t1")
                nc.vector.tensor_tensor(
                    out=t1[:], in0=wf_t[:, :, 0:32],
                    in1=wf_t[:, :, 32:64], op=OP.add)
                t2 = midpool.tile([128, tg, HC // 4], f16, tag="t2")
                nc.vector.tensor_tensor(
                    out=t2[:], in0=t1[:, :, 0:16],
                    in1=t1[:, :, 16:32], op=OP.add)
                lg = lgpool.tile([128, tg, H], f16, tag="lg")
                nc.vector.tensor_tensor(
                    out=lg[:], in0=t2[:, :, 0:8],
                    in1=t2[:, :, 8:16], op=OP.add)
                s["lg"] = lg

            def emit_m(s):
                tg = s["tg"]
                m_t = mtpool.tile([128, tg, PJ], bf16, tag="M")
                nc.scalar.activation(
                    out=m_t[:, :, HC:PJ], in_=s["lg"][:], func=AF.Exp)
                # m = q * ex (ex broadcast over c: middle dim, packed h last)
                nc.vector.tensor_tensor(
                    out=m_t[:, :, 0:HC].rearrange(
                        "p t (c h) -> p t c h", h=H),
                    in0=s["qk"][:, :, 0:HC].rearrange(
                        "p t (c h) -> p t c h", h=H),
                    in1=m_t[:, :, HC:PJ][:, :, None, :].to_broadcast(
                        [128, tg, C, H]),
                    op=OP.mult)
                s["m"] = m_t

            def emit_mm(s):
                gs = s["gs"]
                nb = (gs + 1) // 2
                p_t = ppool.tile([2 * WIN_NODES, nb * PJ], f32)
                for wl in range(gs):
                    x, b = wl % 2, wl // 2
                    prow = slice(x * WIN_NODES, (x + 1) * WIN_NODES)
                    pcols = slice(b * PJ, (b + 1) * PJ)
                    for g in range(TSUB):
                        t = wl * TSUB + g
                        nc.tensor.matmul(
                            p_t[prow, pcols],
                            lhsT=s["S"][:, t, :],
                            rhs=s["m"][:, t, :],
                            start=(g == 0),
                            stop=(g == TSUB - 1),
                        )
                s["psum"] = p_t

            def emit_drain(s):
                gs = s["gs"]
                nb = (gs + 1) // 2
                # drain psum through the (idle) scalar engine, folding the
                # (c,h) -> (h,c) permute into the numerator copy so the
                # vector ops stream contiguous SBUF
                p3 = s["psum"][:].rearrange("p (w j) -> p w j", j=PJ)
                ppn = finpool.tile([2 * WIN_NODES, nb, H, C], f32,
                                   tag="ppn")
                nc.scalar.activation(
                    out=ppn[:].rearrange("p w h c -> p w c h"),
                    in_=p3[:, :, 0:HC].rearrange(
                        "p w (c h) -> p w c h", h=H),
                    func=AF.Copy)
                ppd = finpool.tile([2 * WIN_NODES, nb, H], f32, tag="ppd")
                nc.scalar.activation(
                    out=ppd[:], in_=p3[:, :, HC:PJ], func=AF.Copy)
                s["ppn"], s["ppd"] = ppn, ppd
                s["psum"] = None

            def emit_finish(s, bbase):
                gs = s["gs"]
                nb = (gs + 1) // 2
                rc_t = finpool.tile([2 * WIN_NODES, nb, H], f32, tag="rc")
                nc.vector.reciprocal(rc_t[:], s["ppd"][:])
                d_t = finpool.tile([2 * WIN_NODES, nb, H, C], f32, tag="d")
                nc.vector.tensor_tensor(
                    out=d_t[:],
                    in0=s["ppn"][:],
                    in1=rc_t[:, :, :, None].to_broadcast(
                        [2 * WIN_NODES, nb, H, C]),
                    op=OP.mult)
                o_t = finpool.tile([2 * WIN_NODES, nb, HC], f32, tag="o")
                nc.scalar.activation(
                    o_t[:], d_t[:].rearrange("p w h c -> p w (h c)"),
                    func=AF.Relu)
                nc.sync.dma_start(
                    out=out[:, bbase * HC:(bbase + nb) * HC].rearrange(
                        "p (w c) -> p w c", c=HC),
                    in_=o_t[:],
                )

            gbases, bbases = [], []
            gb = bb = 0
            for gs, nb in zip(group_sizes, nbs):
                gbases.append(gb)
                bbases.append(bb)
                gb += 128 * gs * TSUB
                bb += nb

            ng = len(group_sizes)

            def load(i):
                return emit_load(i, gbases[i], group_sizes[i])

            # software pipeline with 3-group lookahead: the epilogue of
            # group i is emitted two iterations after its matmuls, loads
            # run three ahead, so the DVE always has logits work queued
            # while the PE finishes earlier groups
            st = [None] * ng
            st[0] = load(0)
            emit_logits(st[0])
            if ng > 1:
                st[1] = load(1)
            if ng > 2:
                st[2] = load(2)
            for i in range(ng):
                if i + 3 < ng:
                    st[i + 3] = load(i + 3)
                emit_m(st[i])
                emit_mm(st[i])
                if i + 1 < ng:
                    emit_logits(st[i + 1])
                if i >= 2:
                    emit_drain(st[i - 2])
                    emit_finish(st[i - 2], bbases[i - 2])
                    st[i - 2] = None
            for i in range(max(0, ng - 2), ng):
                emit_drain(st[i])
                emit_finish(st[i], bbases[i])

    nc.finalize()
    return nc


def _host_arrays(query, key, attn_kernel, targets):
    import ml_dtypes

    packs, order, node_start = pack_windows(targets)
    perms, rels, node_order, n_win, n_slots, group_sizes = build_slots(
        packs, order, node_start)

    # (c,h)-major column permutation: new col c*8+h <- orig col h*8+c
    colperm = (np.arange(HC).reshape(C, H).T).reshape(-1)
    wrow_1 = np.asarray(attn_kernel, dtype=np.float16).reshape(-1)  # A[c,h]
    wrow = np.tile(wrow_1, (128, 1))

    q16 = query[:, colperm].astype(np.float16)
    k16 = key[:, colperm].astype(np.float16)
    iota = np.arange(WIN_NODES, dtype=np.int16)
    in_maps = []
    for c in range(N_CORES):
        qkc = np.zeros((n_slots, MW), dtype=np.float16)
        pc = perms[c]
        used = rels[c] >= 0
        qkc[used, :HC] = q16[pc[used]]
        qkc[used, HC:] = k16[pc[used]]
        # one-hot selector in fp8 (1.0 = 0x38): pure index metadata
        sel_u8 = np.where(rels[c][:, None] == iota[None, :],
                          np.uint8(0x38), np.uint8(0))
        in_maps.append({
            "qk": qkc,
            "sel": sel_u8.view(ml_dtypes.float8_e4m3),
            "wrow": wrow,
        })
    return in_maps, node_order, n_win, n_slots, group_sizes


TRACE = False          # set by test harness to capture an NTFF profile
TRACE_CORES = None
LAST_RESULTS = None    # BassKernelResults of the most recent run


def kernel(query, key, attn_kernel, targets):
    global LAST_RESULTS
    query = np.asarray(query, dtype=np.float32)
    key = np.asarray(key, dtype=np.float32)
    attn_kernel = np.asarray(attn_kernel, dtype=np.float32)
    targets = np.asarray(targets, dtype=np.int32)

    _ensure_imports()
    from concourse.bass_utils import run_bass_kernel_spmd

    in_maps, node_order, n_win, n_slots, group_sizes = _host_arrays(
        query, key, attn_kernel, targets)
    nc = build_nc(n_slots, n_win, group_sizes)
    res = run_bass_kernel_spmd(
        nc, in_maps, list(range(N_CORES)),
        trace=TRACE, trace_cores=TRACE_CORES,
    )
    LAST_RESULTS = res

    # unscramble: psum partition-half x + column block b -> window w
    out = np.zeros((N_NODES, HC), dtype=np.float32)
    nbs = [(gs + 1) // 2 for gs in group_sizes]
    for c in range(N_CORES):
        oc = res.results[c]["out"]  # [64, nb_total*HC]
        bb = 0
        wbase = 0
        for gi, (gs, nb) in enumerate(zip(group_sizes, nbs)):
            for b in range(nb):
                for x in range(2):
                    w = wbase + b * 2 + x
                    if w >= wbase + gs:
                        continue
                    rows = node_order[c, w * WIN_NODES:(w + 1) * WIN_NODES]
                    valid = rows >= 0
                    blk = oc[x * WIN_NODES:(x + 1) * WIN_NODES,
                             (bb + b) * HC:(bb + b + 1) * HC]
                    out[rows[valid]] = blk[valid]
            bb += nb
            wbase += gs

    deg = np.bincount(targets, minlength=N_NODES)
    out[deg == 0] = 0.0
    return out
